# revision 34
# baseline (speedup 1.0000x reference)
"""Trainium2 Bass kernel for nn_Decoder_63720134804045.

Data-parallel over batch: 8 cores x 4 batches. Feature-major (transposed)
activation layout on-chip: X^T [D on partitions, rows free]. LayerNorm
affine is folded into W2/W3 on host; LN stats via ones-matmuls on PE;
neighbor-leaf term computed as 5 shifted matmuls over masked leaf
embeddings built with one-hot matmuls on device.
"""
import sys
sys.path.insert(0, '/opt/trn_rl_repo')
from contextlib import ExitStack

import numpy as np

import concourse.bass as bass
import concourse.tile as tile
from concourse import bacc, mybir
from concourse._compat import with_exitstack
from concourse.bass_utils import run_bass_kernel_spmd
from concourse.masks import make_identity

F32 = mybir.dt.float32
F32R = mybir.dt.float32r
I32 = mybir.dt.int32
AF = mybir.ActivationFunctionType
ALU = mybir.AluOpType

B, S, D, V = 32, 64, 768, 50
MAXD, LC = 5, 3
NN = 31                 # heap nodes
NSLOT = 63
NCORES = 8
BL = B // NCORES        # 4 local batches
T = BL * S              # 256 tokens per core
TP = S + 2 * LC         # 70 padded tokens per batch
KC = D // 128           # 6 feature chunks
NROWS = NN * T          # 7936 node-rows per core
NT128 = NROWS // 128    # 62
SHIFTS = [-3, -2, -1, 1, 2]
LEAFK = 256             # max leaf rows per shift block (8 nodes x 32)
EPS = 1e-5

_CACHE = {}


def _build_nc(loop_n=None):
    nc = bacc.Bacc("TRN2", target_bir_lowering=False, debug=False,
                   num_devices=NCORES)
    dt = nc.dram_tensor
    ins = dict(
        memT=dt("memT", [128, KC * T], F32, kind="ExternalInput"),
        idx=dt("idx", [128, NT128], I32, kind="ExternalInput"),
        exm=dt("exm", [128, NT128], F32, kind="ExternalInput"),
        tgtm=dt("tgtm", [15, BL * TP], F32, kind="ExternalInput"),
        W1=dt("W1", [D, D], F32R, kind="ExternalInput"),
        W2=dt("W2", [D, D], F32R, kind="ExternalInput"),
        W3=dt("W3", [D, D], F32R, kind="ExternalInput"),
        Wout=dt("Wout", [D, V], F32R, kind="ExternalInput"),
        biases=dt("biases", [128, 4 * KC], F32, kind="ExternalInput"),
        lemb=dt("lemb", [V, 32], F32, kind="ExternalInput"),
        leafW=dt("leafW", [128, 5 * 2 * D], F32R, kind="ExternalInput"),
        vrow=dt("vrow", [1, 2 * D], F32R, kind="ExternalInput"),
        femb=dt("femb", [20000, D], F32, kind="ExternalInput"),
    )
    out_d = dt("out", [NROWS, V], F32, kind="ExternalOutput")
    aps = {k: v.ap() for k, v in ins.items()}
    with tile.TileContext(nc) as tc:
        if loop_n is None:
            _kernel_body(tc, aps, out_d.ap())
        else:
            with tc.For_i(0, loop_n, 1):
                _kernel_body(tc, aps, out_d.ap())
    nc.compile()
    return nc


@with_exitstack
def _kernel_body(ctx: ExitStack, tc: tile.TileContext, ins, out_d):
    nc = tc.nc
    pw = ctx.enter_context(tc.tile_pool(name="pw", bufs=1))
    p_add = ctx.enter_context(tc.tile_pool(name="p_add", bufs=1))
    p_embT = ctx.enter_context(tc.tile_pool(name="p_embT", bufs=2))
    p_gth = ctx.enter_context(tc.tile_pool(name="p_gth", bufs=3))
    p_act = ctx.enter_context(tc.tile_pool(name="p_act", bufs=1))
    p_asb = ctx.enter_context(tc.tile_pool(name="p_asb", bufs=2))
    p_tg = ctx.enter_context(tc.tile_pool(name="p_tg", bufs=3))
    p_sm = ctx.enter_context(tc.tile_pool(name="p_sm", bufs=3))
    p_osb = ctx.enter_context(tc.tile_pool(name="p_osb", bufs=2))
    p_oh = ctx.enter_context(tc.tile_pool(name="p_oh", bufs=1))
    ps = ctx.enter_context(tc.tile_pool(name="ps", bufs=6, space="PSUM"))
    ps_st = ctx.enter_context(tc.tile_pool(name="ps_st", bufs=2, space="PSUM"))

    # ---- persistent SBUF state ----
    memT = pw.tile([128, KC * T], F32)
    nc.sync.dma_start(memT[:], ins["memT"][:])
    idx_sb = pw.tile([128, NT128], I32)
    nc.sync.dma_start(idx_sb[:], ins["idx"][:])
    exm_sb = pw.tile([128, NT128], F32)
    nc.sync.dma_start(exm_sb[:], ins["exm"][:])
    bias_sb = pw.tile([128, 4 * KC], F32)
    nc.sync.dma_start(bias_sb[:], ins["biases"][:])
    lemb_sb = pw.tile([V, 32], F32)
    nc.sync.dma_start(lemb_sb[:], ins["lemb"][:])
    leafW_sb = pw.tile([128, 5 * 2 * D], F32R)
    nc.sync.dma_start(leafW_sb[:], ins["leafW"][:])
    vrow_sb = pw.tile([1, 2 * D], F32R)
    nc.sync.dma_start(vrow_sb[:], ins["vrow"][:])

    Wsb = {}
    for wname in ("W1", "W2", "W3"):
        for kc in range(KC):
            t_ = pw.tile([128, D], F32R, tag=f"{wname}_{kc}")
            nc.sync.dma_start(t_[:], ins[wname][kc * 128:(kc + 1) * 128, :])
            Wsb[(wname, kc)] = t_
    Wout_sb = []
    for kc in range(KC):
        t_ = pw.tile([128, V], F32R, tag=f"wout_{kc}")
        nc.sync.dma_start(t_[:], ins["Wout"][kc * 128:(kc + 1) * 128, :])
        Wout_sb.append(t_)

    ident = pw.tile([128, 128], F32)
    make_identity(nc, ident[:])
    ones_r = pw.tile([1, 128], F32)       # row of ones (K=1 lhsT)
    nc.vector.memset(ones_r[:], 1.0)
    ones_c = pw.tile([128, 1], F32R)      # column of ones (M=1 lhsT)
    ones_rr = pw.tile([1, 128], F32R)     # f32r row of ones
    ones_cf = pw.tile([128, 1], F32)
    nc.vector.memset(ones_cf[:], 1.0)
    nc.vector.tensor_copy(ones_c[:], ones_cf[:])
    nc.vector.tensor_copy(ones_rr[:], ones_r[:])
    iota_i = pw.tile([V, 1], I32)
    nc.gpsimd.iota(iota_i[:], pattern=[[0, 1]], base=0, channel_multiplier=1)
    iota_f = pw.tile([V, 1], F32)
    nc.vector.tensor_copy(iota_f[:], iota_i[:])
    eps_sb = pw.tile([1, 1], F32)
    nc.vector.memset(eps_sb[:], EPS)

    BTP = BL * TP  # 280

    # ---- phase 1: masked leaf-embedding matrix E ----
    # E columns: [E1 | E2 | E3 | E4a | E4b], each BTP wide.
    # group g covers nodes ngrp[g]; node j of group at partitions 32*j.
    E_sb = pw.tile([128, 5 * BTP], F32)
    ngrps = [[0], [1, 2], [3, 4, 5, 6], [7, 8, 9, 10], [11, 12, 13, 14]]
    for g, nodes in enumerate(ngrps):
        psE = ps.tile([128, BTP], F32, space="PSUM", tag="psbig")
        for j, node in enumerate(nodes):
            tg_st = p_oh.tile([1, BTP], F32, tag="tgst")
            nc.sync.dma_start(tg_st[:], ins["tgtm"][node:node + 1, :])
            psT = ps_st.tile([V, BTP], F32, space="PSUM", tag="psst")
            nc.tensor.matmul(
                psT[:], ones_r[0:1, 0:V], tg_st[:],
                start=True, stop=True)
            oh = p_oh.tile([V, BTP], F32, tag="oh")
            nc.vector.tensor_scalar(out=oh[:], in0=psT[:], scalar1=iota_f[:],
                                    scalar2=None, op0=ALU.is_equal)
            nc.tensor.matmul(psE[32 * j:32 * j + 32, :], lemb_sb[:], oh[:],
                             start=True, stop=True, tile_position=(0, 32 * j))
        nc.vector.tensor_copy(
            E_sb[0:32 * len(nodes), g * BTP:(g + 1) * BTP].bitcast(F32R),
            psE[0:32 * len(nodes), :])

    # leaf-shift matmul sources per depth: (E col group, K rows) per kc chunk
    ECHUNKS = {1: [(0, 32)], 2: [(1, 64)], 3: [(2, 128)], 4: [(3, 128), (4, 128)]}

    def fused_layer(src, dst, wname, bias_col, NW, A_=None,
                    m_=None, vcol=None):
        """dst = gelu(W^T src [*A - v (x) m] + b). LN of the previous layer is
        applied in the psum domain: rank-1 -v (x) m rides the accumulation
        (m is available right after the sum-stats matmul) and *A is one DVE
        op per chunk. The A-broadcast matmul is deferred behind the first
        three chunk matmul groups so the PE never idles on the stats chain."""
        def mm_group(mc):
            pl = ps.tile([128, NW], F32, space="PSUM", tag="psbig",
                         name=f"pl_{wname}_{mc}_{NW}")
            for kc in range(KC):
                nc.tensor.matmul(
                    pl[:], Wsb[(wname, kc)][:, mc * 128:(mc + 1) * 128],
                    src[:, kc * NW:(kc + 1) * NW].bitcast(F32R),
                    start=(kc == 0), stop=(kc == KC - 1 and vcol is None))
            if vcol is not None:
                nc.tensor.matmul(
                    pl[:], vrow_sb[0:1, vcol * D + mc * 128:vcol * D + (mc + 1) * 128],
                    m_[:].bitcast(F32R), start=False, stop=True)
            return pl

        def finish(mc, pl, Asb):
            sl = slice(mc * NW, (mc + 1) * NW)
            if Asb is None:
                nc.scalar.activation(
                    dst[:, sl].bitcast(F32R), pl[:], AF.Gelu,
                    bias=bias_sb[:, bias_col * KC + mc:bias_col * KC + mc + 1])
            else:
                tgc = p_tg.tile([128, NW], F32, tag="tg", name=f"tg_{wname}_{mc}_{NW}")
                nc.vector.tensor_mul(tgc[:], pl[:], Asb[:])
                nc.scalar.activation(
                    dst[:, sl].bitcast(F32R), tgc[:], AF.Gelu,
                    bias=bias_sb[:, bias_col * KC + mc:bias_col * KC + mc + 1])

        if vcol is None:
            for mc in range(KC):
                finish(mc, mm_group(mc), None)
            return
        pls = [mm_group(mc) for mc in range(3)]
        pA = ps.tile([128, NW], F32, space="PSUM", tag="psbig",
                     name=f"pA_{wname}_{NW}")
        nc.tensor.matmul(pA[:], ones_rr[0:1, :], A_[:].bitcast(F32R),
                         start=True, stop=True)
        Asb = p_asb.tile([128, NW], F32, tag="Asb")
        nc.scalar.activation(Asb[:], pA[:], AF.Identity)
        for i in range(3):
            finish(i, pls[i], Asb)
            pls.append(mm_group(3 + i))
        for i in range(3, KC):
            finish(i, pls[i], Asb)

    def ln_stats(src, sq, NW):
        """LN stats: returns (A_ = rstd row, m_ = mean row), both [1, NW]."""
        for mc in range(KC):
            if mc % 2 == 0:
                nc.vector.tensor_mul(sq[:, mc * NW:(mc + 1) * NW].bitcast(F32R),
                                     src[:, mc * NW:(mc + 1) * NW],
                                     src[:, mc * NW:(mc + 1) * NW])
            else:
                nc.scalar.activation(sq[:, mc * NW:(mc + 1) * NW].bitcast(F32R),
                                     src[:, mc * NW:(mc + 1) * NW], AF.Square)
        pss = ps_st.tile([1, NW], F32, space="PSUM", tag="psst")
        for kc in range(KC):
            nc.tensor.matmul(pss[0:1, :], ones_c[:, 0:1],
                             src[:, kc * NW:(kc + 1) * NW].bitcast(F32R),
                             start=(kc == 0), stop=(kc == KC - 1))
        psq = ps_st.tile([1, NW], F32, space="PSUM", tag="psst")
        for kc in range(KC):
            nc.tensor.matmul(psq[0:1, :], ones_c[:, 0:1],
                             sq[:, kc * NW:(kc + 1) * NW].bitcast(F32R),
                             start=(kc == 0), stop=(kc == KC - 1))
        m = p_sm.tile([1, NW], F32, tag="sm")
        nc.vector.tensor_scalar(out=m[:].bitcast(F32R), in0=pss[0:1, :],
                                scalar1=1.0 / D, scalar2=None, op0=ALU.mult)
        msq = p_sm.tile([1, NW], F32, tag="sm")
        nc.vector.tensor_mul(msq[:], m[:], m[:])
        v = p_sm.tile([1, NW], F32, tag="sm")
        nc.vector.scalar_tensor_tensor(out=v[:], in0=psq[0:1, :], scalar=1.0 / D,
                                       in1=msq[:], op0=ALU.mult, op1=ALU.subtract)
        sd = p_sm.tile([1, NW], F32, tag="sm")
        nc.scalar.activation(sd[:], v[:], AF.Sqrt, bias=eps_sb[0:1, 0:1])
        A_ = p_sm.tile([1, NW], F32, tag="sm")
        with nc.allow_low_precision(reason="fp32r rounding of LN rstd"):
            nc.vector.reciprocal(A_[:].bitcast(F32R), sd[:])
        return A_, m

    # ---- main depth loop ----
    for d in range(MAXD):
        lo = 2 ** d - 1
        cnt = 2 ** d
        # add_d = memT (+ OL^T + leaf_b for d>0), chunk-major [128, KC*T]
        if d == 0:
            add_t = memT
        else:
            add_t = p_add.tile([128, KC * T], F32, tag="add")
            for mc in range(KC):
                pol = ps.tile([128, T], F32, space="PSUM", tag="psbig")
                first = True
                nmm = len(SHIFTS) * len(ECHUNKS[d])
                i = 0
                for n, o in enumerate(SHIFTS):
                    for kci, (eg, K) in enumerate(ECHUNKS[d]):
                        lw = leafW_sb[0:K, (n * 2 + kci) * D + mc * 128:(n * 2 + kci) * D + mc * 128 + 128]
                        rhs = (E_sb[0:K, eg * BTP:(eg + 1) * BTP]
                               .rearrange("k (b t) -> k b t", t=TP)
                               [:, :, LC + o:LC + o + S])
                        i += 1
                        nc.tensor.matmul(pol[:], lw, rhs.bitcast(F32R),
                                         start=first, stop=(i == nmm))
                        first = False
                nc.scalar.activation(add_t[:, mc * T:(mc + 1) * T], pol[:], AF.Identity,
                                     bias=bias_sb[:, 3 * KC + mc:3 * KC + mc + 1])
                nc.vector.tensor_add(add_t[:, mc * T:(mc + 1) * T],
                                     add_t[:, mc * T:(mc + 1) * T],
                                     memT[:, mc * T:(mc + 1) * T])

        # node tiles: 2 nodes (512 rows) for d>0, 1 node (256 rows) for d=0
        gtiles = ([[lo]] if d == 0 else
                  [[lo + 2 * i, lo + 2 * i + 1] for i in range(cnt // 2)])
        for gs in gtiles:
            NW = T * len(gs)
            rowbase = gs[0] * T
            ti0 = rowbase // 128
            nsub = NW // 128

            embT = p_embT.tile([128, KC * NW], F32, tag="embT")
            for j in range(nsub):
                gth = p_gth.tile([128, D], F32, tag="gth",
                                 name=f"gth_{rowbase}_{j}")
                nc.gpsimd.indirect_dma_start(
                    out=gth[:], out_offset=None, in_=ins["femb"][:],
                    in_offset=bass.IndirectOffsetOnAxis(
                        ap=idx_sb[:, ti0 + j:ti0 + j + 1], axis=0))
                for grp, glen in ((0, 4), (4, 2)):
                    ptr = ps.tile([128, glen * 128], F32, space="PSUM",
                                  tag="psbig", name=f"ptr_{rowbase}_{j}_{grp}")
                    for ki in range(glen):
                        nc.tensor.transpose(ptr[:, ki * 128:(ki + 1) * 128],
                                            gth[:, (grp + ki) * 128:(grp + ki + 1) * 128],
                                            ident[:])
                    dst = (embT[:].rearrange("p (k w) -> p k w", w=NW)
                           [:, grp:grp + glen, j * 128:(j + 1) * 128])
                    nc.scalar.activation(dst.bitcast(F32R), ptr[:], AF.Identity)

            h = p_act.tile([128, KC * NW], F32, tag="h")
            fused_layer(embT, h, "W1", 0, NW)
            for mc in range(KC):
                for u in range(len(gs)):
                    sl = slice(mc * NW + u * T, mc * NW + (u + 1) * T)
                    nc.vector.tensor_add(h[:, sl].bitcast(F32R), h[:, sl],
                                         add_t[:, mc * T:(mc + 1) * T])
            sq = p_act.tile([128, KC * NW], F32, tag="sq")
            A1, m1 = ln_stats(h, sq, NW)
            x2 = p_act.tile([128, KC * NW], F32, tag="x2")
            fused_layer(h, x2, "W2", 1, NW, A_=A1, m_=m1, vcol=0)
            A2, m2 = ln_stats(x2, sq, NW)
            x3 = p_act.tile([128, KC * NW], F32, tag="sq")
            fused_layer(x2, x3, "W3", 2, NW, A_=A2, m_=m2, vcol=1)

            po = ps.tile([V, NW], F32, space="PSUM", tag="psbig")
            for kc in range(KC):
                nc.tensor.matmul(po[:], Wout_sb[kc][:],
                                 x3[:, kc * NW:(kc + 1) * NW].bitcast(F32R),
                                 start=(kc == 0), stop=(kc == KC - 1))
            eT = p_act.tile([V, NW], F32, tag="eT")
            nc.scalar.activation(eT[:], po[:], AF.Exp)
            for j in range(nsub):
                pt = ps_st.tile([128, V], F32, space="PSUM", tag="psst")
                nc.tensor.transpose(pt[:], eT[0:V, j * 128:(j + 1) * 128],
                                    ident[0:V, 0:V])
                ssum = p_sm.tile([128, 1], F32, tag="smc")
                nc.vector.reduce_sum(ssum[:], pt[:], axis=mybir.AxisListType.X)
                rm = p_sm.tile([128, 1], F32, tag="smc")
                nc.vector.reciprocal(rm[:], ssum[:])
                nc.vector.tensor_mul(rm[:], rm[:],
                                     exm_sb[:, ti0 + j:ti0 + j + 1])
                osb = p_osb.tile([128, V], F32, tag="osb")
                nc.vector.tensor_scalar(out=osb[:], in0=pt[:], scalar1=rm[:],
                                        scalar2=None, op0=ALU.mult)
                nc.sync.dma_start(
                    out_d[rowbase + j * 128:rowbase + (j + 1) * 128, :], osb[:])


def _host_prep(inputs):
    mem = np.asarray(inputs["memory"], np.float32)
    seqlen = np.asarray(inputs["seq_length"])
    tgt = np.asarray(inputs["tgt"])
    fidx = np.asarray(inputs["feat_idx"])
    femb = np.ascontiguousarray(np.asarray(inputs["feat_embs"], np.float32))
    W1 = np.ascontiguousarray(np.asarray(inputs["W1"], np.float32))
    ln_g = np.asarray(inputs["ln_g"], np.float32)
    ln_b = np.asarray(inputs["ln_b"], np.float32)
    W2 = np.asarray(inputs["W2"], np.float32)
    W3 = np.asarray(inputs["W3"], np.float32)
    b1 = np.asarray(inputs["b1"], np.float32)
    b2 = np.asarray(inputs["b2"], np.float32)
    b3 = np.asarray(inputs["b3"], np.float32)
    Wout = np.ascontiguousarray(np.asarray(inputs["Wout"], np.float32))
    lemb = np.ascontiguousarray(np.asarray(inputs["leaf_emb"], np.float32))
    lW = np.asarray(inputs["leaf_W"], np.float32)
    lb = np.asarray(inputs["leaf_b"], np.float32)

    W2f = np.ascontiguousarray(ln_g[:, None] * W2)
    W3f = np.ascontiguousarray(ln_g[:, None] * W3)
    b2f = (b2 + ln_b @ W2).astype(np.float32)
    b3f = (b3 + ln_b @ W3).astype(np.float32)

    tok_valid = np.arange(S)[None, :] < seqlen[:, None]
    is_slash = (tgt == 0) | (tgt == 1)
    ex = np.zeros((B, S, NN), bool)
    ex[:, :, 0] = tok_valid
    for i in range(1, NN):
        p = (i - 1) // 2
        ex[:, :, i] = ex[:, :, p] & is_slash[:, :, p]

    biases = np.stack([b1.reshape(KC, 128), b2f.reshape(KC, 128),
                       b3f.reshape(KC, 128), lb.reshape(KC, 128)])  # [4,KC,128]
    biases_sb = np.ascontiguousarray(
        biases.reshape(4 * KC, 128).T)  # [128, 4*KC]

    leafW_p = np.zeros((128, 5 * 2 * D), np.float32)
    for n in range(5):
        for kci in range(2):
            blk = lW[n * 480 + kci * 128: n * 480 + (kci + 1) * 128, :]
            leafW_p[:, (n * 2 + kci) * D:(n * 2 + kci + 1) * D] = blk

    vrow = np.concatenate([-W2f.sum(0), -W3f.sum(0)]).reshape(1, 2 * D).astype(np.float32)
    shared = dict(W1=W1, W2=W2f, W3=W3f, Wout=Wout, biases=biases_sb,
                  lemb=lemb, leafW=leafW_p, femb=femb, vrow=vrow)

    in_maps = []
    for c in range(NCORES):
        bsl = slice(c * BL, (c + 1) * BL)
        memT = np.ascontiguousarray(
            mem[bsl].reshape(T, D).T.reshape(KC, 128, T)
            .transpose(1, 0, 2).reshape(128, KC * T))
        idx_flat = fidx[bsl].transpose(2, 0, 1).reshape(NROWS)
        idx_sb = np.ascontiguousarray(
            idx_flat.reshape(NT128, 128).T).astype(np.int32)
        exf = ex[bsl].transpose(2, 0, 1).reshape(NROWS).astype(np.float32)
        exm_sb = np.ascontiguousarray(exf.reshape(NT128, 128).T)
        tgtm = np.where(ex[bsl, :, :15], tgt[bsl, :, :15], -1).astype(np.float32)
        tgtm_p = np.full((15, BL, TP), -1.0, np.float32)
        tgtm_p[:, :, LC:LC + S] = tgtm.transpose(2, 0, 1)
        tgtm_sb = np.ascontiguousarray(tgtm_p.reshape(15, BL * TP))
        in_maps.append(dict(memT=memT, idx=idx_sb, exm=exm_sb, tgtm=tgtm_sb,
                            **shared))
    return in_maps


def kernel(**inputs):
    if "nc" not in _CACHE:
        _CACHE["nc"] = _build_nc()
    nc = _CACHE["nc"]
    in_maps = _host_prep(inputs)
    res = run_bass_kernel_spmd(nc, in_maps, core_ids=list(range(NCORES)))
    out = np.zeros((B, S, NSLOT, V), np.float32)
    for c in range(NCORES):
        dev = res.results[c]["out"]                      # [NROWS, V]
        out[c * BL:(c + 1) * BL, :, :NN, :] = (
            dev.reshape(NN, BL, S, V).transpose(1, 2, 0, 3))
    return out
